# revision 37
# baseline (speedup 1.0000x reference)
"""Trainium2 Bass kernel for nn_Conv1dFFTInt8.

The reference computes, per (b, o):
    out[b,o,0] = ifft(fft(x) . fft(w) summed over cin)[0] + bias[o]
By the circular correlation theorem this collapses to a plain dot product:
    out[b,o] = sum_{i,n} x[b,i,n] * w[o,i,(L-n) % L] + bias[o]

So the whole problem is a GEMM: [B, CIN*L] @ [CIN*L, COUT] with a 524288-deep
contraction. We shard the contraction (CIN) across 8 cores (16 channels each).

v2 (DoubleRow): each core runs 256 fp8e4 DoubleRow matmuls. DoubleRow
processes TWO 128-deep k-tiles per instruction (the PE holds two weight rows
per cell and double-pumps the moving stream), so the PE consumes a pair of
k-tiles in ~53ns instead of ~107ns - the PE drops off the critical path and
the kernel is purely DMA-stream-bound.

DoubleRow requires BOTH operands in fp8e4/e5. x in e4m3 alone costs rel err
2.6e-2 (> the 2e-2 gate), so the stationary operand packs an e4m3 RESIDUAL
correction into the idle M columns: lhsT = [K=128, 2, 32] where cols 0-15
are x_hi = e4m3(x) and cols 16-31 are x_r = e4m3(x - x_hi). One w stream
then computes main + correction simultaneously (PSUM rows 16-31 hold the
correction); host sums the halves. Measured rel err: 7.4e-4.

Streams per core: w 8.39 MB + x 2.10 MB (hi+residual) = 10.5 MB against a
~360-420 GB/s per-core HBM stream, chunked at the 4KB-per-partition
descriptor knee and byte-balanced across the two HWDGE rings (sync/scalar).
Four PE column strips (tile_position cols 0/32/64/96) accumulate in separate
PSUM banks so LDWEIGHTS hides under other strips' matmuls. Warmup dummy MMs
pre-trip the HAM clock up during the (fixed) NEFF preamble; light keepalives
hold it there across chunk waits. Tail: DVE+ACT evacuate strips in parallel,
one 64 KB out DMA whose receipt is not waited (NEFF-end drains cover it).
"""

import numpy as np
import ml_dtypes

import concourse.bass as bass
from concourse import bacc
import concourse.mybir as mybir
from concourse.bass_utils import run_bass_kernel_spmd

B, CIN, COUT, L = 16, 128, 128, 4096
NCORES = 8
CIN_SH = CIN // NCORES          # 16 channels per core
KT = 128                        # contraction depth per k-tile
NKT = CIN_SH * L // KT          # 512 k-tiles per core
NPAIR = NKT // 2                # 256 DoubleRow pairs per core
MST = 2 * B                     # stationary M columns: 16 hi + 16 residual

# --- tunables (A/B config) ---
CFG = dict(
    # DoubleRow ISA restriction (NeuronVerifier check_dual_fp8_restriction):
    # dst.start_partition must be 0, which in bass ties the stationary tile
    # to PE columns 0-31 - so no multi-strip tile_position in DR mode.
    # Multiple PSUM banks at the same partitions do not add LDW overlap, so
    # nstrip stays 1 unless measurement says LDW stalls the pipe.
    nstrip=1,                   # PE column strips (tile_position cols 32*s)
    # Residual coverage: pairs {0..n_wide-1} carry the e4m3 residual
    # correction (stationary [K,2,32]); the rest ride hi-only (stationary
    # [K,2,16]). The device is HBM-bound, so the 384KB/core of skipped
    # residual bytes buy ~1.1us; exact rel err at n_wide=160 is 1.63e-2
    # (host-verified, deterministic inputs; gate 2e-2). Pair 0 is wide so
    # start=True zeroes PSUM rows 0:32; later narrow MMs only touch 0:16.
    n_wide=160,
    # w chunk sizes in pairs; 16 pairs = 4KB/partition. Per-engine SDMA
    # packet cost is bandwidth-flat (4KB=158ns, 8KB=309, 2KB=83), so chunk
    # size does not change engine busy time - fine chunks win because the
    # sem pacing matches the PE and the tail drain stays short. Small tail
    # chunks so the final sem -> last-MM drain is short.
    w_sizes=(16,) * 15 + (8, 8),
    # x chunk sizes in wide-slots (xa tensor) and narrow-slots (xb tensor).
    xa_sizes=(64, 64, 32),
    xb_sizes=(96,),
    # Explicit per-ring FIFO issue order (ring 0=sync queue Q1, 1=scalar
    # queue Q10). Measured: Q10's first byte lands ~2.2us after Q1's, and
    # each ring sustains ~half of the ~410 GB/s aggregate, so ring 0 carries
    # more bytes. Orders are arranged so chunk COMPLETION order matches the
    # PE's need order (w0,w1,...), with each x chunk landing 2-3 chunks
    # before the w chunks that need it - a k-sorted order once made ring 1
    # deliver w12 dead last at 40.6us and forced a 3.6us serial PE drain.
    ring0=(("a", 0), ("w", 0), ("a", 2), ("w", 2), ("w", 4), ("w", 6),
           ("w", 8), ("w", 10), ("w", 12), ("w", 14), ("w", 16)),
    ring1=(("a", 1), ("w", 1), ("b", 0), ("w", 3), ("w", 5), ("w", 7),
           ("w", 9), ("w", 11), ("w", 13), ("w", 15)),
    warmup=45,                  # dummy MMs at PE start to pre-trip HAM and
                                # ride out the p-state ramp; ends right as
                                # w0's data lands
    # The PE runs at ~86% natural utilization against the stream; HAM dips
    # to k=4 during the DMA ramp but recovers by the drain phase, costing
    # nothing at the end (measured: last MM lands 0.65us after the last
    # chunk sem). Dummies issued while the PE is behind add straight to the
    # critical path, so none mid-stream.
    keepalive=(0,) * 17,
    wait_out=False,             # skip waiting for the out DMA receipt
)

TRACE = False                   # set by test.py to profile
LAST_RESULTS = None             # BassKernelResults of the last run

_PROG_CACHE = {}


def _wide_slot(p, n_wide):
    """Wide (hi+residual) pairs are {0..n_wide-1} -> xa slots; narrow
    (hi-only) pairs are {n_wide..255} -> xb slots. Returns (is_wide, slot).
    A wide prefix means PSUM rows 16:32 (the residual half) finalize at
    pair n_wide-1, mid-stream - so their copy+out-DMA overlap the stream.
    """
    if p < n_wide:
        return True, p
    return False, p - n_wide


def _build_program_raw(cfg):
    """Raw bacc implementation: manual semaphores, no TileContext."""
    nstrip = cfg["nstrip"]
    n_wide = cfg["n_wide"]
    w_sizes = cfg["w_sizes"]
    xa_sizes = cfg["xa_sizes"]
    xb_sizes = cfg["xb_sizes"]
    n_narrow = NPAIR - n_wide
    assert sum(w_sizes) == NPAIR
    assert sum(xa_sizes) == n_wide and sum(xb_sizes) == n_narrow
    n_wc = len(w_sizes)
    w_start = np.cumsum([0] + list(w_sizes))  # pair offsets
    xa_start = np.cumsum([0] + list(xa_sizes))
    xb_start = np.cumsum([0] + list(xb_sizes))
    # per w chunk: highest xa/xb chunk index needed (or -1 if none)
    xa_need, xb_need = [], []
    for c in range(n_wc):
        amax = bmax = -1
        for p in range(int(w_start[c]), int(w_start[c + 1])):
            wide, slot = _wide_slot(p, n_wide)
            if wide:
                amax = max(amax, int(np.searchsorted(xa_start, slot,
                                                     side="right")) - 1)
            else:
                bmax = max(bmax, int(np.searchsorted(xb_start, slot,
                                                     side="right")) - 1)
        xa_need.append(amax)
        xb_need.append(bmax)
    rings = {0: cfg["ring0"], 1: cfg["ring1"]}
    issued = sorted(t for r in rings.values() for t in r)
    assert issued == sorted([("w", i) for i in range(n_wc)] +
                            [("a", i) for i in range(len(xa_sizes))] +
                            [("b", i) for i in range(len(xb_sizes))])

    nc = bacc.Bacc("TRN2", target_bir_lowering=False, debug=False,
                   num_devices=NCORES)
    xa_d = nc.dram_tensor("xa", [KT, n_wide, 2, MST], mybir.dt.float8e4,
                          kind="ExternalInput")
    xb_d = nc.dram_tensor("xb", [KT, n_narrow, 2, B], mybir.dt.float8e4,
                          kind="ExternalInput")
    wt_d = nc.dram_tensor("wt", [KT, NPAIR, 2, COUT], mybir.dt.float8e4,
                          kind="ExternalInput")
    out_d = nc.dram_tensor("out", [KT, COUT], mybir.dt.float32,
                           kind="ExternalOutput")

    import contextlib
    with contextlib.ExitStack() as stack:
        ec = stack.enter_context
        # one sem per DMA transfer: with several transfers in flight on the
        # 16 SDMA engines, a single cumulative sem is unsound (fast engines
        # can reach 16*(c+1) before a slow engine lands transfer c).
        s_wc = [ec(nc.semaphore(f"s_w{c}")) for c in range(n_wc)]
        s_ac = [ec(nc.semaphore(f"s_a{c}")) for c in range(len(xa_sizes))]
        s_bc = [ec(nc.semaphore(f"s_b{c}")) for c in range(len(xb_sizes))]
        s_mm = ec(nc.semaphore("s_mm"))
        s_half = ec(nc.semaphore("s_half"))
        s_out = ec(nc.semaphore("s_out"))
        xsa = ec(nc.sbuf_tensor("xsa", [KT, n_wide, 2, MST],
                                mybir.dt.float8e4))
        xsb = ec(nc.sbuf_tensor("xsb", [KT, n_narrow, 2, B],
                                mybir.dt.float8e4))
        ws = ec(nc.sbuf_tensor("ws", [KT, NPAIR, 2, COUT], mybir.dt.float8e4))
        osb = ec(nc.sbuf_tensor("osb", [KT, COUT], mybir.dt.float32))
        osb2 = ec(nc.sbuf_tensor("osb2", [B, COUT], mybir.dt.float32))
        accs = [ec(nc.psum_tensor(f"acc{s}", [KT, COUT], mybir.dt.float32))
                for s in range(nstrip)]
        # narrow pairs accumulate in their own bank so the wide bank is
        # closed (no concurrent PE writes) when scalar reads it mid-stream
        acc2 = ec(nc.psum_tensor("acc2", [KT, COUT], mybir.dt.float32))
        if cfg["warmup"] or any(cfg["keepalive"]):
            junk = ec(nc.sbuf_tensor("junk", [KT, COUT], mybir.dt.float8e4))
            scr = ec(nc.psum_tensor("scr", [KT, COUT], mybir.dt.float32))

        def emit_ring(eng, ring):
            for kind, c in rings[ring]:
                if kind == "a":
                    a, b = int(xa_start[c]), int(xa_start[c + 1])
                    eng.dma_start(xsa[:, a:b, :, :],
                                  xa_d[:, a:b, :, :]).then_inc(s_ac[c], 16)
                elif kind == "b":
                    a, b = int(xb_start[c]), int(xb_start[c + 1])
                    # probe: 15-partition sub-transfers; if the queue's
                    # descriptor->engine rotor restarts per transfer, none
                    # of these touch engine 79 (desc index mod 16 <= 14)
                    for p0 in range(0, KT, 15):
                        p1 = min(p0 + 15, KT)
                        eng.dma_start(
                            xsb[p0:p1, a:b, :, :],
                            xb_d[p0:p1, a:b, :, :]).then_inc(s_bc[c], 16)
                else:
                    a, b = int(w_start[c]), int(w_start[c + 1])
                    eng.dma_start(ws[:, a:b, :, :],
                                  wt_d[:, a:b, :, :]).then_inc(s_wc[c], 16)

        with nc.Block() as block:

            @block.sync
            def _(sync):
                emit_ring(sync, 0)

            @block.scalar
            def _(scalar):
                emit_ring(scalar, 1)
                # single-engine out path (scalar is HWDGE), split in two:
                # the residual half (rows 16:32) finalizes at the last wide
                # pair mid-stream, so its copy + out DMA overlap the stream;
                # only rows 0:16 remain on the post-last-MM critical path.
                # the wide accumulation group (bank acc0, rows 0:32) closes
                # at pair n_wide-1 mid-stream: evacuate + DMA it overlapped
                # with the stream; only the narrow bank (rows 0:16 of acc2,
                # pushed to out rows 32:48) rides the post-last-MM tail.
                scalar.wait_ge(s_half, 1)
                scalar.copy(osb[0:2 * B, :], accs[0][0:2 * B, :])
                scalar.dma_start(out_d[0:2 * B, :],
                                 osb[0:2 * B, :]).then_inc(s_out, 16)
                scalar.wait_ge(s_mm, 1)
                scalar.copy(osb2[0:B, :], acc2[0:B, :])
                scalar.dma_start(out_d[2 * B:3 * B, :],
                                 osb2[0:B, :]).then_inc(s_out, 16)
                if cfg["wait_out"]:
                    scalar.wait_ge(s_out, 32)

            @block.tensor
            def _(tensor):
                def dummy_mms(n):
                    # scratch-bank matmuls: keep the PE busy across DMA waits
                    # so HAM holds the clock up; results are never read
                    for _ in range(n):
                        tensor.matmul(scr[0:B, :], junk[:, 0:B],
                                      junk[:, 0:COUT], start=True, stop=True)

                dummy_mms(cfg["warmup"])
                a_waited = b_waited = -1
                ka = cfg["keepalive"]
                assert len(ka) == n_wc
                for c, chunk in enumerate(w_sizes):
                    dummy_mms(ka[c])
                    tensor.wait_ge(s_wc[c], 16)
                    if xa_need[c] > a_waited:
                        a_waited = xa_need[c]
                        tensor.wait_ge(s_ac[a_waited], 16)
                    if xb_need[c] > b_waited:
                        b_waited = xb_need[c]
                        tensor.wait_ge(s_bc[b_waited],
                                       16 * ((KT + 14) // 15))
                    for j in range(chunk):
                        p = int(w_start[c]) + j
                        wide, slot = _wide_slot(p, n_wide)
                        if wide:
                            lhsT, m, dst = xsa[:, slot, :, :], MST, accs[0]
                            start, stop = (p == 0), (p == n_wide - 1)
                        else:
                            lhsT, m, dst = xsb[:, slot, :, :], B, acc2
                            start, stop = (p == n_wide), (p == NPAIR - 1)
                        mm = tensor.matmul(
                            dst[0:m, :],
                            lhsT,
                            ws[:, p, :, :],
                            start=start,
                            stop=stop,
                            perf_mode=mybir.MatmulPerfMode.DoubleRow,
                            tile_position=(0, 0),
                        )
                        if p == n_wide - 1:
                            mm.then_inc(s_half, 2)
                        if p == NPAIR - 1:
                            mm.then_inc(s_mm, 2)



    nc.compile()
    return nc


def _get_program(cfg):
    key = repr(sorted(cfg.items()))
    if key not in _PROG_CACHE:
        _PROG_CACHE[key] = _build_program_raw(cfg)
    return _PROG_CACHE[key]


def _pack_operand(arr_k_major, ncols):
    """[n_pairs*2*KT, ncols] contraction-major -> [KT, n_pairs, 2, ncols]
    where sb[r, p, i, c] = arr[(2p + i)*KT + r, c]."""
    nkt = arr_k_major.shape[0] // KT
    a = arr_k_major.reshape(nkt, KT, ncols).transpose(1, 0, 2)
    return np.ascontiguousarray(a).reshape(KT, nkt // 2, 2, ncols)


def kernel(x, weight, bias):
    import os
    if not TRACE:
        # profiling needs an NTFF hook this image lacks; never trace here
        os.environ["BASS_NEVER_TRACE"] = "1"
    else:
        os.environ.pop("BASS_NEVER_TRACE", None)
    x = np.asarray(x, dtype=np.float32)
    weight = np.asarray(weight, dtype=np.float32)
    bias = np.asarray(bias, dtype=np.float32)

    cfg = dict(CFG)
    nc = _get_program(cfg)
    nstrip = cfg["nstrip"]

    # w_rev[o,i,n] = weight[o,i,(L-n) % L]
    idx = (L - np.arange(L)) % L
    wrev = weight[:, :, idx]

    # split x into e4m3 hi + e4m3 residual (DoubleRow needs fp8e4 operands;
    # hi alone would cost 2.6e-2 rel err, hi+residual costs 7.4e-4)
    x_hi8 = x.astype(ml_dtypes.float8_e4m3)
    x_r8 = (x - x_hi8.astype(np.float32)).astype(ml_dtypes.float8_e4m3)

    n_wide = cfg["n_wide"]
    wide_pairs = list(range(n_wide))
    narrow_pairs = list(range(n_wide, NPAIR))
    in_maps = []
    for c in range(NCORES):
        i0 = c * CIN_SH
        wsl = wrev[:, i0:i0 + CIN_SH, :].reshape(COUT, CIN_SH * L)
        wt = _pack_operand(wsl.T.astype(ml_dtypes.float8_e4m3), COUT)
        # [K_total, 32] = hi cols 0-15, residual cols 16-31
        xh = x_hi8[:, i0:i0 + CIN_SH, :].reshape(B, CIN_SH * L).T
        xr = x_r8[:, i0:i0 + CIN_SH, :].reshape(B, CIN_SH * L).T
        xsl = np.concatenate([xh, xr], axis=1)
        # wide pairs carry hi+residual; narrow pairs carry hi only
        k_of = lambda plist: np.concatenate(
            [np.arange(p * 2 * KT, (p + 1) * 2 * KT) for p in plist])
        xa = _pack_operand(xsl[k_of(wide_pairs)], MST)
        xb = _pack_operand(xsl[k_of(narrow_pairs)][:, :B], B)
        in_maps.append({"xa": xa, "xb": xb, "wt": wt})

    global LAST_RESULTS
    res = run_bass_kernel_spmd(nc, in_maps, core_ids=list(range(NCORES)),
                               trace=TRACE)
    LAST_RESULTS = res

    acc = np.zeros((B, COUT), np.float32)
    for c in range(NCORES):
        o = res.results[c]["out"]
        # rows 0:16 wide-hi, 16:32 residual, 32:48 narrow-hi
        acc += o[0:B, :] + o[B:2 * B, :] + o[2 * B:3 * B, :]
    out = acc + bias[None, :]
    return out[:, :, None].astype(np.float32)
